# revision 25
# baseline (speedup 1.0000x reference)
"""DeepseekV2-Lite decoder layer on 8 Trainium2 NeuronCores.

Sharding (chosen to minimize per-call host->device input bytes, which is the
dominant cost in this environment — inputs re-ship every call at ~1GB/s/core):
  - attention: tensor-parallel over heads (2 heads/core, all tokens);
    AllGather of (x_norm^T, c_norm^T, k_pe^T) feeds q/k/v projections;
    row-parallel o_proj partials + ReduceScatter back to token-parallel.
  - MLP: tensor-parallel over the intermediate dim (1368 rows/core of
    Wg/Wu/Wd, padded to 1408): AllGather(y_norm^T) -> gate/up/down partials
    for all tokens -> ReduceScatter(add) back to token-parallel.
Per-core shipped inputs ~24MB (vs ~141MB data-parallel MLP). hid/weights/
tables all bf16; output bf16 (cast to f32 on host). All matmuls bf16 with
fp32 PSUM accumulation.
"""
import math
import sys

sys.path.insert(0, "/opt/trn_rl_repo")

import numpy as np
import ml_dtypes

import concourse.bass as bass
import concourse.mybir as mybir
import concourse.tile as tile
from concourse.masks import make_identity

# ---------------------------------------------------------------------------
# Patch: the hardware CTRL instruction supports only one sync-wait slot, but
# kernels with collectives need several on the final Tile drain. Split the
# excess onto SP nops emitted right after the drain, before the sem-clear.
# ---------------------------------------------------------------------------
from concourse.vector_clock import ScopedClock


def _drain_and_barrier_split(self, tick_clock, wait_clock):
    drain_inst = self.nc.sync.drain()
    wait_clock.add_sem_waits(
        drain_inst.ins, ScopedClock({None: tick_clock.global_clock})
    )
    si = drain_inst.ins.sync_info
    if si is not None and len(si.on_wait) > 1:
        waits = list(si.on_wait)
        drain_inst.ins.sync_info = mybir.SyncInfo(
            on_wait=waits[:1], on_update=list(si.on_update)
        )
        for w in waits[1:]:
            nop = self.nc.sync.nop(nofuse=True, hint="drain_wait_overflow")
            nop.ins.sync_info = mybir.SyncInfo(on_wait=[w], on_update=[])
    self.nc.all_engine_barrier()
    assert self.sems is not None
    popped = self.nc._tile_sem_poison_stack.pop()
    assert popped is self._sem_poison
    self.nc.clear_and_free_semaphores(list(self.sems.allocated().values()))
    self.nc.all_engine_barrier()


tile.TileContext._drain_and_barrier = _drain_and_barrier_split

# ---------------------------------------------------------------------------
# Several instruction encodings (DMA, CTRL) accept only one sync-wait slot.
# Split every multi-wait instruction at BIR-serialization time: excess waits
# move onto same-engine NoOps inserted immediately before the instruction.
# ---------------------------------------------------------------------------
import orjson as _orjson

if not getattr(bass.Bass, "_wait_split_patched", False):
    bass.Bass._orig_to_json_bytes = bass.Bass.to_json_bytes
    bass.Bass._wait_split_patched = True
_orig_to_json_bytes = bass.Bass._orig_to_json_bytes


def _to_json_bytes_split(self):
    data = _orjson.loads(_orig_to_json_bytes(self))
    ctr = 0
    for f in data.get("functions", []):
        for bb in f.get("basic_blocks", f.get("blocks", [])):
            insts = bb.get("instructions", [])
            out = []
            for inst in insts:
                si = inst.get("sync_info")
                if si and len(si.get("on_wait") or []) > 1:
                    waits = si["on_wait"]
                    for w in waits[:-1]:
                        ctr += 1
                        out.append({
                            "debug": inst.get("debug", 0),
                            "engine": inst["engine"],
                            "ins": [], "name": f"I-ws{ctr}",
                            "opcode": "NoOp", "outs": [],
                            "sync_info": {"on_update": [], "on_wait": [w]},
                            "text_hint": "wait_split",
                        })
                    si["on_wait"] = [waits[-1]]
                out.append(inst)
            bb["instructions"] = out
    return _orjson.dumps(data)


bass.Bass.to_json_bytes = _to_json_bytes_split

# ---------------------------------------------------------------------------
FULL_CFG = dict(
    B=2, S=2048, HID=2048, H=16, D_NOPE=128, D_ROPE=64, D_V=128, KV=512,
    INTER=10944, N_CORES=8,
)
EPS = 1e-6
MAX_POS, BASE, FACTOR, ORIG_MAX = 8192, 10000.0, 40.0, 4096
BETA_FAST, BETA_SLOW, MSCALE, MSCALE_ALL = 32, 1, 0.707, 0.707

BF = mybir.dt.bfloat16
F32 = mybir.dt.float32
AX = mybir.AxisListType
AF = mybir.ActivationFunctionType


def _derived(cfg):
    d = dict(cfg)
    d["T_TOT"] = cfg["B"] * cfg["S"]
    d["T_LOC"] = d["T_TOT"] // cfg["N_CORES"]
    d["HPC"] = cfg["H"] // cfg["N_CORES"]
    d["KH"] = cfg["HID"] // 128
    d["KC"] = cfg["KV"] // 128
    d["TSUB"] = d["T_LOC"] // 128
    d["NCH"] = d["T_TOT"] // d["T_LOC"]
    # intermediate dim padded so each core gets ILJ chunks of 128
    n128 = (cfg["INTER"] + 127) // 128
    d["ILJ"] = (n128 + cfg["N_CORES"] - 1) // cfg["N_CORES"]   # chunks per core
    d["ILC"] = d["ILJ"] * 128                                   # inter rows per core
    d["INTER_PAD"] = d["ILC"] * cfg["N_CORES"]
    d["QTILES_B"] = cfg["S"] // 512
    d["KB_B"] = cfg["S"] // 128
    d["DQ"] = cfg["D_NOPE"] + cfg["D_ROPE"]
    # xnT + cnT + kpeT + cosLT + sinLT rows
    d["AGROWS"] = cfg["HID"] + cfg["KV"] + 2 * cfg["D_ROPE"]
    return d


# ---------------------------------------------------------------------------
def build_kernel(cfg):
    c = _derived(cfg)
    N = c["N_CORES"]
    HID, KV, DR, DN, DV = c["HID"], c["KV"], c["D_ROPE"], c["D_NOPE"], c["D_V"]
    TL, TT = c["T_LOC"], c["T_TOT"]
    KH, KC, TSUB, NCH = c["KH"], c["KC"], c["TSUB"], c["NCH"]
    HPC, DQ = c["HPC"], c["DQ"]
    QT_B, KB_B = c["QTILES_B"], c["KB_B"]
    B = c["B"]
    ILJ = c["ILJ"]
    HR = DR // 2
    AGR = c["AGROWS"]

    nc = bass.Bass()
    hid_e = nc.dram_tensor("hid", [TL, HID], BF, kind="ExternalInput")
    wqT_e = nc.dram_tensor("wqT", [HID, HPC * DQ], BF, kind="ExternalInput")
    wkvaT_e = nc.dram_tensor("wkvaT", [HID, KV + DR], BF, kind="ExternalInput")
    wbnT_e = nc.dram_tensor("wbnT", [KV, HPC * DN], BF, kind="ExternalInput")
    wbvT_e = nc.dram_tensor("wbvT", [KV, HPC * DV], BF, kind="ExternalInput")
    woT_e = nc.dram_tensor("woT", [HPC * DV, HID], BF, kind="ExternalInput")
    wg_e = nc.dram_tensor("wgp", [128, KH * ILJ * 128], BF, kind="ExternalInput")
    wu_e = nc.dram_tensor("wup", [128, KH * ILJ * 128], BF, kind="ExternalInput")
    wd_e = nc.dram_tensor("wdp", [128, ILJ * HID], BF, kind="ExternalInput")
    cosL_e = nc.dram_tensor("cosL", [TL, HR], BF, kind="ExternalInput")
    sinL_e = nc.dram_tensor("sinL", [TL, HR], BF, kind="ExternalInput")
    out_e = nc.dram_tensor("out", [TL, HID], BF, kind="ExternalOutput")
    probe = cfg.get("probe", False)
    if probe:
        p_agin_e = nc.dram_tensor("p_agin", [AGR, TL], BF, kind="ExternalOutput")
        p_x2_e = nc.dram_tensor("p_x2", [TL, HID], F32, kind="ExternalOutput")

    with tile.TileContext(nc) as tc:
        with (
            tc.tile_pool(name="dram", bufs=1, space="DRAM") as dram,
            tc.tile_pool(name="const", bufs=1) as const,
        ):
            agin = dram.tile([AGR, TL], BF, tag="agin", name="agin")
            agout = dram.tile([N * AGR, TL], BF, addr_space="Shared", tag="agout", name="agout")
            rs_in = dram.tile([TT, HID], BF, tag="rsin", name="rsin")
            rs_out = dram.tile([TL, HID], BF, tag="rsout", name="rsout")
            agin2 = dram.tile([HID, TL], BF, tag="agin2", name="agin2")
            agout2 = dram.tile([N * HID, TL], BF, addr_space="Shared", tag="agout2", name="agout2")
            rs2_in = dram.tile([TT, HID], BF, tag="rs2in", name="rs2in")
            rs2_out = dram.tile([TL, HID], BF, tag="rs2out", name="rs2out")
            x2_dr = dram.tile([TL, HID], F32, tag="x2dr", name="x2dr")

            # stage MLP weights host->device-DRAM on the Act HWDGE queue so the
            # slow external-input link streams them during the whole attention
            # block; phase 8 then reads them from fast internal DRAM.
            wg_dr = dram.tile([128, KH * ILJ * 128], BF, tag="wgdr", name="wgdr")
            nc.scalar.dma_start(wg_dr[:], wg_e[:])
            wu_dr = dram.tile([128, KH * ILJ * 128], BF, tag="wudr", name="wudr")
            nc.scalar.dma_start(wu_dr[:], wu_e[:])
            wd_dr = dram.tile([128, ILJ * HID], BF, tag="wddr", name="wddr")
            nc.scalar.dma_start(wd_dr[:], wd_e[:])

            ident = const.tile([128, 128], BF, tag="ident", name="ident")
            make_identity(nc, ident)
            eps_sb = const.tile([128, 1], F32, tag="eps", name="eps")
            nc.vector.memset(eps_sb[:], EPS)
            # mask[p, x] = 1.0 if x >= p + 384 else 0.0, generated on device
            mask_sb = const.tile([128, 896], BF, tag="mask", name="mask")
            nc.gpsimd.memset(mask_sb[:], 1.0)
            nc.gpsimd.affine_select(
                out=mask_sb[:], in_=mask_sb[:],
                compare_op=mybir.AluOpType.is_ge, fill=0.0,
                base=-384, pattern=[[1, 896]], channel_multiplier=-1,
            )
            cosT_sb = const.tile([HR, TT], F32, tag="cosT", name="cosT")
            sinT_sb = const.tile([HR, TT], F32, tag="sinT", name="sinT")
            cosL_sb = const.tile([128, TSUB, HR], F32, tag="cosL", name="cosL")
            nc.gpsimd.dma_start(cosL_sb[:], cosL_e.rearrange("(a p) r -> p a r", p=128))
            sinL_sb = const.tile([128, TSUB, HR], F32, tag="sinL", name="sinL")
            nc.gpsimd.dma_start(sinL_sb[:], sinL_e.rearrange("(a p) r -> p a r", p=128))

            # ============ phases 0-1: rms1, x^T, ckv, rms(c), rope(k_pe) =====
            with (
                tc.tile_pool(name="xnTp", bufs=1) as xnTp,
                tc.tile_pool(name="p0", bufs=2) as p0,
                tc.tile_pool(name="p01ps", bufs=2, space="PSUM") as p01ps,
            ):
                xnT = [xnTp.tile([128, TL], BF, tag=f"xnT{k}", name=f"xnT{k}") for k in range(KH)]
                xn_sb = []
                for t in range(TSUB):
                    ht = p0.tile([128, HID], BF, tag="hid0", name="hid0")
                    nc.sync.dma_start(ht[:], hid_e[t * 128:(t + 1) * 128, :])
                    sq = p0.tile([128, HID], F32, tag="sq", name="sq")
                    nc.vector.tensor_mul(sq[:], ht[:], ht[:])
                    ssum = p0.tile([128, 1], F32, tag="ssum", name="ssum")
                    nc.vector.reduce_sum(out=ssum[:], in_=sq[:], axis=AX.X)
                    rs = p0.tile([128, 1], F32, tag="rs", name="rs")
                    nc.scalar.activation(rs[:], ssum[:], AF.Sqrt, scale=1.0 / HID, bias=eps_sb[:])
                    nc.vector.reciprocal(rs[:], rs[:])
                    xt = p0.tile([128, HID], BF, tag="xn", name="xn", bufs=TSUB)
                    nc.vector.tensor_scalar_mul(xt[:], ht[:], rs[:])
                    xn_sb.append(xt)
                for t in range(TSUB):
                    for k in range(KH):
                        ps = p01ps.tile([128, 128], BF, tag="tr", name="tr")
                        nc.tensor.transpose(ps[:], xn_sb[t][:, k * 128:(k + 1) * 128], ident[:])
                        nc.scalar.copy(xnT[k][:, t * 128:(t + 1) * 128], ps[:])
                for k in range(KH):
                    nc.sync.dma_start(agin[k * 128:(k + 1) * 128, :], xnT[k][:])

                # phase 1
                wkva_sb = [p0.tile([128, KV + DR], BF, tag=f"wkva{k}", name=f"wkva{k}") for k in range(KH)]
                for k in range(KH):
                    nc.sync.dma_start(wkva_sb[k][:], wkvaT_e[k * 128:(k + 1) * 128, :])
                cnT_sb = [p0.tile([128, TL], BF, tag=f"cnT{j}", name=f"cnT{j}") for j in range(KC)]
                kpeT_loc = p0.tile([DR, TL], BF, tag="kpeT_loc", name="kpeT_loc")
                for t in range(TSUB):
                    ps_c = p01ps.tile([128, KV], F32, tag="psc", name="psc")
                    ps_p = p01ps.tile([128, DR], F32, tag="psp", name="psp")
                    for k in range(KH):
                        lq = xnT[k][:, t * 128:(t + 1) * 128]
                        nc.tensor.matmul(ps_c[:], lq, wkva_sb[k][:, :KV],
                                         start=(k == 0), stop=(k == KH - 1))
                        nc.tensor.matmul(ps_p[:], lq, wkva_sb[k][:, KV:],
                                         start=(k == 0), stop=(k == KH - 1))
                    sq = p0.tile([128, KV], F32, tag="sqc", name="sqc")
                    nc.scalar.activation(sq[:], ps_c[:], AF.Square)
                    ssum = p0.tile([128, 1], F32, tag="ssumc", name="ssumc")
                    nc.vector.reduce_sum(out=ssum[:], in_=sq[:], axis=AX.X)
                    rs = p0.tile([128, 1], F32, tag="rsc", name="rsc")
                    nc.scalar.activation(rs[:], ssum[:], AF.Sqrt, scale=1.0 / KV, bias=eps_sb[:])
                    nc.vector.reciprocal(rs[:], rs[:])
                    cn = p0.tile([128, KV], BF, tag="cn", name="cn")
                    nc.vector.tensor_scalar_mul(cn[:], ps_c[:], rs[:])
                    kp = p0.tile([128, DR], BF, tag="kp", name="kp")
                    a = p0.tile([128, HR], F32, tag="ra", name="ra")
                    b = p0.tile([128, HR], F32, tag="rb", name="rb")
                    cosl = cosL_sb[:, t, :]
                    sinl = sinL_sb[:, t, :]
                    nc.vector.tensor_mul(a[:], ps_p[:, :HR], cosl)
                    nc.vector.tensor_mul(b[:], ps_p[:, HR:], sinl)
                    nc.vector.tensor_sub(kp[:, :HR], a[:], b[:])
                    nc.vector.tensor_mul(a[:], ps_p[:, HR:], cosl)
                    nc.vector.tensor_mul(b[:], ps_p[:, :HR], sinl)
                    nc.vector.tensor_add(kp[:, HR:], a[:], b[:])
                    for j in range(KC):
                        ps = p01ps.tile([128, 128], BF, tag="tr", name="tr")
                        nc.tensor.transpose(ps[:], cn[:, j * 128:(j + 1) * 128], ident[:])
                        nc.scalar.copy(cnT_sb[j][:, t * 128:(t + 1) * 128], ps[:])
                    ps = p01ps.tile([128, 128], BF, tag="tr", name="tr")
                    nc.tensor.transpose(ps[:DR, :], kp[:], ident[:])
                    nc.scalar.copy(kpeT_loc[:, t * 128:(t + 1) * 128], ps[:DR, :])
                for j in range(KC):
                    nc.sync.dma_start(agin[HID + j * 128:HID + (j + 1) * 128, :], cnT_sb[j][:])
                nc.sync.dma_start(agin[HID + KV:HID + KV + DR, :], kpeT_loc[:])
                # ride cosL^T/sinL^T along the AllGather to avoid shipping
                # broadcast cosT/sinT tables to every core
                cosLT = p0.tile([HR, TL], BF, tag="cosLT", name="cosLT")
                sinLT = p0.tile([HR, TL], BF, tag="sinLT", name="sinLT")
                for t in range(TSUB):
                    cb = p0.tile([128, HR], BF, tag="cb", name="cb")
                    nc.scalar.copy(cb[:], cosL_sb[:, t, :])
                    ps = p01ps.tile([128, 128], BF, tag="tr", name="tr")
                    nc.tensor.transpose(ps[:HR, :], cb[:], ident[:])
                    nc.scalar.copy(cosLT[:, t * 128:(t + 1) * 128], ps[:HR, :])
                    sb2 = p0.tile([128, HR], BF, tag="sb2", name="sb2")
                    nc.scalar.copy(sb2[:], sinL_sb[:, t, :])
                    ps = p01ps.tile([128, 128], BF, tag="tr", name="tr")
                    nc.tensor.transpose(ps[:HR, :], sb2[:], ident[:])
                    nc.scalar.copy(sinLT[:, t * 128:(t + 1) * 128], ps[:HR, :])
                nc.sync.dma_start(agin[HID + KV + DR:HID + KV + DR + HR, :], cosLT[:])
                nc.sync.dma_start(agin[HID + KV + DR + HR:HID + KV + 2 * DR, :], sinLT[:])

            # ============ phase 2: AllGather ================================
            nc.gpsimd.collective_compute(
                "AllGather", mybir.AluOpType.bypass,
                replica_groups=[list(range(N))],
                ins=[agin.opt()], outs=[agout.opt()],
            )

            # assemble full cos/sin tables (f32) from the gathered slices
            for ch in range(NCH):
                base = ch * AGR + HID + KV + DR
                nc.gpsimd.dma_start(
                    cosT_sb[:, ch * TL:(ch + 1) * TL], agout[base:base + HR, :])
                nc.gpsimd.dma_start(
                    sinT_sb[:, ch * TL:(ch + 1) * TL], agout[base + HR:base + DR, :])

            if probe:
                with tc.tile_pool(name="prb0", bufs=2) as prb0:
                    for r in range(0, AGR, 128):
                        w = min(128, AGR - r)
                        pt_ = prb0.tile([128, TL], BF, tag="pgt", name="pgt")
                        nc.sync.dma_start(pt_[:w, :], agin[r:r + w, :])
                        nc.sync.dma_start(p_agin_e[r:r + w, :], pt_[:w, :])

            # ============ phases 3-5: attention ==============================
            with tc.tile_pool(name="asb", bufs=1) as asb:
                qnT = [asb.tile([128, TT], BF, tag=f"qnT{h}", name=f"qnT{h}") for h in range(HPC)]
                qpT = [asb.tile([DR, TT], BF, tag=f"qpT{h}", name=f"qpT{h}") for h in range(HPC)]
                knT = [asb.tile([128, TT], BF, tag=f"knT{h}", name=f"knT{h}") for h in range(HPC)]
                kpeT = asb.tile([DR, TT], BF, tag="kpeT", name="kpeT")
                v_sb = [asb.tile([128, TT // 128, DV + 4], BF, tag=f"v{h}", name=f"v{h}")
                        for h in range(HPC)]
                atT = [asb.tile([128, TT], BF, tag=f"atT{h}", name=f"atT{h}") for h in range(HPC)]

                with (
                    tc.tile_pool(name="p4w", bufs=1) as p4w,
                    tc.tile_pool(name="p4x", bufs=1) as p4x,
                    tc.tile_pool(name="p4", bufs=2) as p4,
                    tc.tile_pool(name="p4ps", bufs=2, space="PSUM") as p4ps,
                ):
                    wq_sb = [p4w.tile([128, HPC * DQ], BF, tag=f"wq{k}", name=f"wq{k}") for k in range(KH)]
                    for k in range(KH):
                        nc.sync.dma_start(wq_sb[k][:], wqT_e[k * 128:(k + 1) * 128, :])
                    wbn_sb = [p4w.tile([128, HPC * DN], BF, tag=f"wbn{j}", name=f"wbn{j}") for j in range(KC)]
                    wbv_sb = [p4w.tile([128, HPC * DV], BF, tag=f"wbv{j}", name=f"wbv{j}") for j in range(KC)]
                    for j in range(KC):
                        nc.sync.dma_start(wbn_sb[j][:], wbnT_e[j * 128:(j + 1) * 128, :])
                        nc.sync.dma_start(wbv_sb[j][:], wbvT_e[j * 128:(j + 1) * 128, :])

                    for ch in range(NCH):
                        nc.sync.dma_start(
                            kpeT[:, ch * TL:(ch + 1) * TL],
                            agout[ch * AGR + HID + KV: ch * AGR + HID + KV + DR, :])

                    for ch in range(NCH):
                        xch = []
                        for k in range(KH):
                            xt = p4x.tile([128, TL], BF, tag="xch", name="xch", bufs=KH + 4)
                            nc.sync.dma_start(
                                xt[:], agout[ch * AGR + k * 128: ch * AGR + (k + 1) * 128, :])
                            xch.append(xt)
                        cs = slice(ch * TL, (ch + 1) * TL)
                        for h in range(HPC):
                            ps_n = p4ps.tile([128, TL], F32, tag="qn", name="qn")
                            ps_p = p4ps.tile([DR, TL], F32, tag="qp", name="qp")
                            off = h * DQ
                            for k in range(KH):
                                nc.tensor.matmul(ps_n[:], wq_sb[k][:, off:off + DN], xch[k][:],
                                                 start=(k == 0), stop=(k == KH - 1))
                            for k in range(KH):
                                nc.tensor.matmul(ps_p[:], wq_sb[k][:, off + DN:off + DQ], xch[k][:],
                                                 start=(k == 0), stop=(k == KH - 1))
                            nc.scalar.copy(qnT[h][:, cs], ps_n[:])
                            a = p4.tile([HR, TL], F32, tag="qa", name="qa")
                            b = p4.tile([HR, TL], F32, tag="qb", name="qb")
                            cosc = cosT_sb[:, cs]
                            sinc = sinT_sb[:, cs]
                            nc.vector.tensor_mul(a[:], ps_p[:HR, :], cosc)
                            nc.vector.tensor_mul(b[:], ps_p[HR:, :], sinc)
                            nc.vector.tensor_sub(qpT[h][:HR, cs], a[:], b[:])
                            nc.vector.tensor_mul(a[:], ps_p[HR:, :], cosc)
                            nc.vector.tensor_mul(b[:], ps_p[:HR, :], sinc)
                            nc.vector.tensor_add(qpT[h][HR:, cs], a[:], b[:])

                    for ch in range(NCH):
                        cch = []
                        for j in range(KC):
                            ct = p4x.tile([128, TL], BF, tag="cch", name="cch", bufs=KC + 2)
                            nc.sync.dma_start(
                                ct[:], agout[ch * AGR + HID + j * 128: ch * AGR + HID + (j + 1) * 128, :])
                            cch.append(ct)
                        cs = slice(ch * TL, (ch + 1) * TL)
                        for h in range(HPC):
                            ps_k = p4ps.tile([128, TL], F32, tag="kn", name="kn")
                            for j in range(KC):
                                nc.tensor.matmul(ps_k[:], wbn_sb[j][:, h * DN:(h + 1) * DN], cch[j][:],
                                                 start=(j == 0), stop=(j == KC - 1))
                            nc.scalar.copy(knT[h][:, cs], ps_k[:])
                            for j4 in range(TL // 128):
                                ps_v = p4ps.tile([128, DV], F32, tag="pv", name="pv")
                                for j in range(KC):
                                    nc.tensor.matmul(ps_v[:], cch[j][:, j4 * 128:(j4 + 1) * 128],
                                                     wbv_sb[j][:, h * DV:(h + 1) * DV],
                                                     start=(j == 0), stop=(j == KC - 1))
                                kbt = ch * (TL // 128) + j4
                                nc.scalar.copy(v_sb[h][:, kbt, :DV], ps_v[:])
                                nc.vector.memset(v_sb[h][:, kbt, DV:DV + 1], 1.0)

                # ---------------- phase 5: attention -------------------------
                with (
                    tc.tile_pool(name="p5ps", bufs=2, space="PSUM") as p5ps,
                    tc.tile_pool(name="p5pv", bufs=2, space="PSUM") as p5pv,
                    tc.tile_pool(name="p5", bufs=2) as p5,
                    tc.tile_pool(name="prb", bufs=1) as prb,
                ):
                    for b in range(B):
                        for h in range(HPC):
                            for qt in range(QT_B):
                                qs = slice(b * cfg["S"] + qt * 512, b * cfg["S"] + qt * 512 + 512)
                                nkb = 4 * qt + 4
                                pt = []
                                for kb in range(nkb):
                                    kbg = b * KB_B + kb
                                    ks = slice(kbg * 128, kbg * 128 + 128)
                                    ps_s = p5ps.tile([128, 512], F32, tag="ps_s", name="ps_s")
                                    nc.tensor.matmul(ps_s[:], knT[h][:, ks], qnT[h][:, qs],
                                                     start=True, stop=False)
                                    nc.tensor.matmul(ps_s[:], kpeT[:, ks], qpT[h][:, qs],
                                                     start=False, stop=True)
                                    pb = prb.tile([128, 512], BF, tag="pb", name="pb", bufs=KB_B + 4)
                                    nc.scalar.activation(pb[:], ps_s[:], AF.Exp)
                                    delta = kb * 128 - qt * 512
                                    if delta >= 0:
                                        nc.vector.tensor_mul(
                                            pb[:], pb[:], mask_sb[:, 384 - delta:896 - delta])
                                    pt.append(pb)
                                for q4 in range(4):
                                    ps_av = p5pv.tile([128, DV + 4], F32, tag="ps_av", name="ps_av")
                                    for kb in range(nkb):
                                        kbt = b * KB_B + kb
                                        nc.tensor.matmul(
                                            ps_av[:, :DV + 1],
                                            pt[kb][:, q4 * 128:(q4 + 1) * 128],
                                            v_sb[h][:, kbt, :DV + 1],
                                            start=(kb == 0), stop=(kb == nkb - 1))
                                    recip = p5.tile([128, 1], F32, tag="recip", name="recip")
                                    nc.vector.reciprocal(recip[:], ps_av[:, DV:DV + 1])
                                    at = p5.tile([128, DV], BF, tag="at", name="at")
                                    nc.vector.tensor_scalar_mul(at[:], ps_av[:, :DV], recip[:])
                                    ps_t = p5ps.tile([128, 128], BF, tag="ps_t", name="ps_t")
                                    nc.tensor.transpose(ps_t[:DV, :], at[:], ident[:])
                                    qg = (b * cfg["S"] + qt * 512) // 128 + q4
                                    nc.scalar.copy(atT[h][:DV, qg * 128:(qg + 1) * 128], ps_t[:DV, :])

                # ============ phase 5b: row-parallel o_proj partials =============
                with (
                    tc.tile_pool(name="p6w", bufs=1) as p6w,
                    tc.tile_pool(name="p6", bufs=4) as p6,
                    tc.tile_pool(name="p6ps", bufs=4, space="PSUM") as p6ps,
                ):
                    wo_sb = [p6w.tile([128, HID], BF, tag=f"wo{j}", name=f"wo{j}") for j in range(HPC)]
                    for j in range(HPC):
                        nc.sync.dma_start(wo_sb[j][:], woT_e[j * DV:(j + 1) * DV, :])
                    for tq in range(TT // 128):
                        for nsl in range(HID // 512):
                            ps_o = p6ps.tile([128, 512], F32, tag="ps_o", name="ps_o")
                            for j in range(HPC):
                                nc.tensor.matmul(ps_o[:], atT[j][:DV, tq * 128:(tq + 1) * 128],
                                                 wo_sb[j][:, nsl * 512:(nsl + 1) * 512],
                                                 start=(j == 0), stop=(j == HPC - 1))
                            ob = p6.tile([128, 512], BF, tag="ob", name="ob")
                            nc.scalar.copy(ob[:], ps_o[:])
                            nc.sync.dma_start(
                                rs_in[tq * 128:(tq + 1) * 128, nsl * 512:(nsl + 1) * 512], ob[:])

            # ============ phase 6: ReduceScatter =============================
            nc.gpsimd.collective_compute(
                "ReduceScatter", mybir.AluOpType.add,
                replica_groups=[list(range(N))],
                ins=[rs_in.opt()], outs=[rs_out.opt()],
            )

            # ============ phase 7: residual, rms2, yT ========================
            with (
                tc.tile_pool(name="p7a", bufs=1) as p7a,
                tc.tile_pool(name="p7", bufs=2) as p7,
                tc.tile_pool(name="p7ps2", bufs=4, space="PSUM") as p7ps2,
            ):
                ynT = [p7a.tile([128, TL], BF, tag=f"ynT{k}", name=f"ynT{k}") for k in range(KH)]
                for t in range(TSUB):
                    hid_r = p7.tile([128, HID], BF, tag="hidr", name="hidr")
                    nc.sync.dma_start(hid_r[:], hid_e[t * 128:(t + 1) * 128, :])
                    rs_sb = p7.tile([128, HID], BF, tag="rssb", name="rssb")
                    nc.sync.dma_start(rs_sb[:], rs_out[t * 128:(t + 1) * 128, :])
                    x2 = p7.tile([128, HID], F32, tag="x2", name="x2")
                    nc.vector.tensor_add(x2[:], rs_sb[:], hid_r[:])
                    nc.sync.dma_start(x2_dr[t * 128:(t + 1) * 128, :], x2[:])
                    if probe:
                        nc.sync.dma_start(p_x2_e[t * 128:(t + 1) * 128, :], x2[:])
                    sq = p7.tile([128, HID], F32, tag="sq", name="sq")
                    nc.vector.tensor_mul(sq[:], x2[:], x2[:])
                    ssum = p7.tile([128, 1], F32, tag="ssum", name="ssum")
                    nc.vector.reduce_sum(out=ssum[:], in_=sq[:], axis=AX.X)
                    rs = p7.tile([128, 1], F32, tag="rs", name="rs")
                    nc.scalar.activation(rs[:], ssum[:], AF.Sqrt, scale=1.0 / HID, bias=eps_sb[:])
                    nc.vector.reciprocal(rs[:], rs[:])
                    yt = p7.tile([128, HID], BF, tag="yn", name="yn")
                    nc.vector.tensor_scalar_mul(yt[:], x2[:], rs[:])
                    for k in range(KH):
                        ps = p7ps2.tile([128, 128], BF, tag="tr", name="tr")
                        nc.tensor.transpose(ps[:], yt[:, k * 128:(k + 1) * 128], ident[:])
                        nc.scalar.copy(ynT[k][:, t * 128:(t + 1) * 128], ps[:])
                for k in range(KH):
                    nc.sync.dma_start(agin2[k * 128:(k + 1) * 128, :], ynT[k][:])

            # ============ phase 8a: AllGather(yT) ============================
            nc.gpsimd.collective_compute(
                "AllGather", mybir.AluOpType.bypass,
                replica_groups=[list(range(N))],
                ins=[agin2.opt()], outs=[agout2.opt()],
            )

            # ============ phase 8b: TP MLP over local intermediate slice =====
            with (
                tc.tile_pool(name="p8w", bufs=1) as p8w,
                tc.tile_pool(name="p8y", bufs=2) as p8y,
                tc.tile_pool(name="p8", bufs=3) as p8,
                tc.tile_pool(name="p8ps", bufs=2, space="PSUM") as p8ps,
            ):
                wg_sb = p8w.tile([128, KH * ILJ * 128], BF, tag="wg", name="wg")
                nc.sync.dma_start(wg_sb[:], wg_dr[:])
                wu_sb = p8w.tile([128, KH * ILJ * 128], BF, tag="wu", name="wu")
                nc.sync.dma_start(wu_sb[:], wu_dr[:])
                wd_sb = p8w.tile([128, ILJ * HID], BF, tag="wd", name="wd")
                nc.sync.dma_start(wd_sb[:], wd_dr[:])
                for ch in range(NCH):
                    yblk = []
                    for k in range(KH):
                        yb = p8y.tile([128, TL], BF, tag="yb", name="yb", bufs=KH + 4)
                        nc.sync.dma_start(
                            yb[:], agout2[ch * HID + k * 128: ch * HID + (k + 1) * 128, :])
                        yblk.append(yb)
                    hblk = []
                    for j in range(ILJ):
                        ps_g = p8ps.tile([128, TL], F32, tag="psg", name="psg")
                        ps_u = p8ps.tile([128, TL], F32, tag="psu", name="psu")
                        for k in range(KH):
                            nc.tensor.matmul(ps_g[:], wg_sb[:, (k * ILJ + j) * 128:(k * ILJ + j + 1) * 128],
                                             yblk[k][:], start=(k == 0), stop=(k == KH - 1))
                        for k in range(KH):
                            nc.tensor.matmul(ps_u[:], wu_sb[:, (k * ILJ + j) * 128:(k * ILJ + j + 1) * 128],
                                             yblk[k][:], start=(k == 0), stop=(k == KH - 1))
                        sig = p8.tile([128, TL], BF, tag="sig", name="sig")
                        nc.scalar.activation(sig[:], ps_g[:], AF.Silu)
                        ht = p8.tile([128, TL], BF, tag="ht", name="ht", bufs=ILJ + 2)
                        nc.vector.tensor_mul(ht[:], sig[:], ps_u[:])
                        hblk.append(ht)
                    for tq in range(TL // 128):
                        for f in range(HID // 512):
                            ps_d = p8ps.tile([128, 512], F32, tag="psd", name="psd")
                            for j in range(ILJ):
                                nc.tensor.matmul(ps_d[:], hblk[j][:, tq * 128:(tq + 1) * 128],
                                                 wd_sb[:, j * HID + f * 512: j * HID + f * 512 + 512],
                                                 start=(j == 0), stop=(j == ILJ - 1))
                            ob = p8.tile([128, 512], BF, tag="ob", name="ob")
                            nc.scalar.copy(ob[:], ps_d[:])
                            nc.sync.dma_start(
                                rs2_in[ch * TL + tq * 128: ch * TL + (tq + 1) * 128,
                                       f * 512:(f + 1) * 512], ob[:])

            # ============ phase 8c: ReduceScatter(down partials) =============
            nc.gpsimd.collective_compute(
                "ReduceScatter", mybir.AluOpType.add,
                replica_groups=[list(range(N))],
                ins=[rs2_in.opt()], outs=[rs2_out.opt()],
            )

            # ============ phase 9: final residual ============================
            with tc.tile_pool(name="p9", bufs=2) as p9:
                for t in range(TSUB):
                    x2r = p9.tile([128, HID], F32, tag="x2r", name="x2r")
                    nc.sync.dma_start(x2r[:], x2_dr[t * 128:(t + 1) * 128, :])
                    mr = p9.tile([128, HID], BF, tag="mr", name="mr")
                    nc.sync.dma_start(mr[:], rs2_out[t * 128:(t + 1) * 128, :])
                    ot = p9.tile([128, HID], BF, tag="ot", name="ot")
                    nc.vector.tensor_add(ot[:], x2r[:], mr[:])
                    nc.sync.dma_start(out_e[t * 128:(t + 1) * 128, :], ot[:])
    return nc


# ---------------------------------------------------------------------------
# Host-side prep
# ---------------------------------------------------------------------------
def _yarn_tables(position_ids, d_rope):
    ar = np.arange(0, d_rope, 2, dtype=np.float32) / d_rope
    freq_extra = 1.0 / BASE ** ar
    freq_inter = 1.0 / (FACTOR * BASE ** ar)

    def corr_dim(num_rot):
        return d_rope * math.log(ORIG_MAX / (num_rot * 2 * math.pi)) / (2 * math.log(BASE))

    low = max(math.floor(corr_dim(BETA_FAST)), 0)
    high = min(math.ceil(corr_dim(BETA_SLOW)), d_rope - 1)
    hi = high + 0.001 if low == high else high
    ramp = np.clip((np.arange(d_rope // 2, dtype=np.float32) - low) / (hi - low), 0.0, 1.0)
    inv_freq_mask = 1.0 - ramp
    inv_freq = freq_inter * (1 - inv_freq_mask) + freq_extra * inv_freq_mask

    def get_mscale(s, m):
        return 1.0 if s <= 1 else 0.1 * m * math.log(s) + 1.0

    ms = get_mscale(FACTOR, MSCALE) / get_mscale(FACTOR, MSCALE_ALL)
    pos = np.asarray(position_ids).reshape(-1).astype(np.float32)
    fr = np.outer(pos, inv_freq)
    return (np.cos(fr) * ms).astype(np.float32), (np.sin(fr) * ms).astype(np.float32)


def _deint_perm(d):
    p = np.empty(d, np.int64)
    p[:d // 2] = 2 * np.arange(d // 2)
    p[d // 2:] = 2 * np.arange(d // 2) + 1
    return p


def prep_inputs(cfg, hidden_states, position_ids, Wq, Wkva, w_kvln, Wkvb, Wo,
                Wg, Wu, Wd, w_ln1, w_ln2):
    c = _derived(cfg)
    N, HPC = c["N_CORES"], c["HPC"]
    HID, KV, DR, DN, DV, DQ = c["HID"], c["KV"], c["D_ROPE"], c["D_NOPE"], c["D_V"], c["DQ"]
    TL, TT, KH = c["T_LOC"], c["T_TOT"], c["KH"]
    ILJ, ILC = c["ILJ"], c["ILC"]
    bf = ml_dtypes.bfloat16

    hid_flat = np.ascontiguousarray(hidden_states.reshape(TT, HID)).astype(bf)
    perm = _deint_perm(DR)
    scale = np.float32(DQ ** -0.5)

    Wq = Wq * w_ln1[None, :] * scale
    Wqh = Wq.reshape(cfg["H"], DQ, HID)
    Wqh = np.concatenate([Wqh[:, :DN], Wqh[:, DN:][:, perm]], axis=1)
    Wkva = Wkva * w_ln1[None, :]
    Wkva = np.concatenate([Wkva[:KV], Wkva[KV:][perm]], axis=0)
    wkvaT = np.ascontiguousarray(Wkva.T).astype(bf)
    Wkvb = Wkvb * w_kvln[None, :]
    Wkvbh = Wkvb.reshape(cfg["H"], DN + DV, KV)
    WoT_f = np.ascontiguousarray(Wo.T, dtype=np.float32)
    IP = c["INTER_PAD"]
    WgT = np.zeros((HID, IP), np.float32)
    WgT[:, :cfg["INTER"]] = (Wg * w_ln2[None, :]).T
    WuT = np.zeros((HID, IP), np.float32)
    WuT[:, :cfg["INTER"]] = (Wu * w_ln2[None, :]).T
    WdT = np.zeros((IP, HID), np.float32)
    WdT[:cfg["INTER"], :] = Wd.T

    cos_f, sin_f = _yarn_tables(position_ids, DR)

    in_maps = []
    for core in range(N):
        h0 = core * HPC
        wqT = np.ascontiguousarray(
            Wqh[h0:h0 + HPC].transpose(2, 0, 1).reshape(HID, HPC * DQ)).astype(bf)
        wbnT = np.ascontiguousarray(
            Wkvbh[h0:h0 + HPC, :DN].transpose(2, 0, 1).reshape(KV, HPC * DN)).astype(bf)
        wbvT = np.ascontiguousarray(
            Wkvbh[h0:h0 + HPC, DN:].transpose(2, 0, 1).reshape(KV, HPC * DV)).astype(bf)
        sl = slice(core * TL, (core + 1) * TL)
        isl = slice(core * ILC, (core + 1) * ILC)
        # [128 hid_in_part, KH*ILJ*128]: chunk (k,j) at cols (k*ILJ+j)*128
        wgp = np.ascontiguousarray(
            WgT[:, isl].reshape(KH, 128, ILJ * 128).transpose(1, 0, 2)
            .reshape(128, KH * ILJ * 128)).astype(bf)
        wup = np.ascontiguousarray(
            WuT[:, isl].reshape(KH, 128, ILJ * 128).transpose(1, 0, 2)
            .reshape(128, KH * ILJ * 128)).astype(bf)
        # [128 inter_part, ILJ*HID]: chunk j at cols j*HID
        wdp = np.ascontiguousarray(
            WdT[isl, :].reshape(ILJ, 128, HID).transpose(1, 0, 2)
            .reshape(128, ILJ * HID)).astype(bf)
        in_maps.append({
            "hid": hid_flat[sl],
            "wqT": wqT,
            "wkvaT": wkvaT,
            "wbnT": wbnT,
            "wbvT": wbvT,
            "woT": np.ascontiguousarray(WoT_f[h0 * DV:(h0 + HPC) * DV]).astype(bf),
            "wgp": wgp,
            "wup": wup,
            "wdp": wdp,
            "cosL": np.ascontiguousarray(cos_f[sl]).astype(bf),
            "sinL": np.ascontiguousarray(sin_f[sl]).astype(bf),
        })
    return in_maps


def run_cfg(cfg, nc, inputs_dict):
    from concourse.bass_utils import run_bass_kernel_spmd
    c = _derived(cfg)
    in_maps = prep_inputs(cfg, **inputs_dict)
    res = run_bass_kernel_spmd(nc, in_maps, list(range(cfg["N_CORES"])))
    out = np.concatenate(
        [res.results[i]["out"] for i in range(cfg["N_CORES"])], axis=0)
    return out.reshape(cfg["B"], cfg["S"], cfg["HID"]).astype(np.float32), res


_NC_CACHE = {}


def kernel(hidden_states, position_ids, Wq, Wkva, w_kvln, Wkvb, Wo, Wg, Wu, Wd,
           w_ln1, w_ln2):
    cfg = FULL_CFG
    if "full" not in _NC_CACHE:
        _NC_CACHE["full"] = build_kernel(cfg)
    out, _ = run_cfg(cfg, _NC_CACHE["full"], dict(
        hidden_states=np.asarray(hidden_states, np.float32),
        position_ids=np.asarray(position_ids),
        Wq=np.asarray(Wq, np.float32), Wkva=np.asarray(Wkva, np.float32),
        w_kvln=np.asarray(w_kvln, np.float32), Wkvb=np.asarray(Wkvb, np.float32),
        Wo=np.asarray(Wo, np.float32), Wg=np.asarray(Wg, np.float32),
        Wu=np.asarray(Wu, np.float32), Wd=np.asarray(Wd, np.float32),
        w_ln1=np.asarray(w_ln1, np.float32), w_ln2=np.asarray(w_ln2, np.float32)))
    return out



# revision 27
# speedup vs baseline: 1.1916x; 1.1916x over previous
"""DeepseekV2-Lite decoder layer on 8 Trainium2 NeuronCores.

Sharding (chosen to minimize per-call host->device input bytes, which is the
dominant cost in this environment — inputs re-ship every call at ~1GB/s/core):
  - attention: tensor-parallel over heads (2 heads/core, all tokens);
    AllGather of (x_norm^T, c_norm^T, k_pe^T) feeds q/k/v projections;
    row-parallel o_proj partials + ReduceScatter back to token-parallel.
  - MLP: tensor-parallel over the intermediate dim (1368 rows/core of
    Wg/Wu/Wd, padded to 1408): AllGather(y_norm^T) -> gate/up/down partials
    for all tokens -> ReduceScatter(add) back to token-parallel.
Per-core shipped inputs ~24MB (vs ~141MB data-parallel MLP). hid/weights/
tables all bf16; output bf16 (cast to f32 on host). All matmuls bf16 with
fp32 PSUM accumulation.
"""
import math
import sys

sys.path.insert(0, "/opt/trn_rl_repo")

import numpy as np
import ml_dtypes

import concourse.bass as bass
import concourse.mybir as mybir
import concourse.tile as tile
from concourse.masks import make_identity

# ---------------------------------------------------------------------------
# Patch: the hardware CTRL instruction supports only one sync-wait slot, but
# kernels with collectives need several on the final Tile drain. Split the
# excess onto SP nops emitted right after the drain, before the sem-clear.
# ---------------------------------------------------------------------------
from concourse.vector_clock import ScopedClock


def _drain_and_barrier_split(self, tick_clock, wait_clock):
    drain_inst = self.nc.sync.drain()
    wait_clock.add_sem_waits(
        drain_inst.ins, ScopedClock({None: tick_clock.global_clock})
    )
    si = drain_inst.ins.sync_info
    if si is not None and len(si.on_wait) > 1:
        waits = list(si.on_wait)
        drain_inst.ins.sync_info = mybir.SyncInfo(
            on_wait=waits[:1], on_update=list(si.on_update)
        )
        for w in waits[1:]:
            nop = self.nc.sync.nop(nofuse=True, hint="drain_wait_overflow")
            nop.ins.sync_info = mybir.SyncInfo(on_wait=[w], on_update=[])
    self.nc.all_engine_barrier()
    assert self.sems is not None
    popped = self.nc._tile_sem_poison_stack.pop()
    assert popped is self._sem_poison
    self.nc.clear_and_free_semaphores(list(self.sems.allocated().values()))
    self.nc.all_engine_barrier()


tile.TileContext._drain_and_barrier = _drain_and_barrier_split

# ---------------------------------------------------------------------------
# Several instruction encodings (DMA, CTRL) accept only one sync-wait slot.
# Split every multi-wait instruction at BIR-serialization time: excess waits
# move onto same-engine NoOps inserted immediately before the instruction.
# ---------------------------------------------------------------------------
import orjson as _orjson

if not getattr(bass.Bass, "_wait_split_patched", False):
    bass.Bass._orig_to_json_bytes = bass.Bass.to_json_bytes
    bass.Bass._wait_split_patched = True
_orig_to_json_bytes = bass.Bass._orig_to_json_bytes


def _to_json_bytes_split(self):
    data = _orjson.loads(_orig_to_json_bytes(self))
    ctr = 0
    for f in data.get("functions", []):
        for bb in f.get("basic_blocks", f.get("blocks", [])):
            insts = bb.get("instructions", [])
            out = []
            for inst in insts:
                si = inst.get("sync_info")
                if si and len(si.get("on_wait") or []) > 1:
                    waits = si["on_wait"]
                    for w in waits[:-1]:
                        ctr += 1
                        out.append({
                            "debug": inst.get("debug", 0),
                            "engine": inst["engine"],
                            "ins": [], "name": f"I-ws{ctr}",
                            "opcode": "NoOp", "outs": [],
                            "sync_info": {"on_update": [], "on_wait": [w]},
                            "text_hint": "wait_split",
                        })
                    si["on_wait"] = [waits[-1]]
                out.append(inst)
            bb["instructions"] = out
    return _orjson.dumps(data)


bass.Bass.to_json_bytes = _to_json_bytes_split

# ---------------------------------------------------------------------------
FULL_CFG = dict(
    B=2, S=2048, HID=2048, H=16, D_NOPE=128, D_ROPE=64, D_V=128, KV=512,
    INTER=10944, N_CORES=8,
)
EPS = 1e-6
MAX_POS, BASE, FACTOR, ORIG_MAX = 8192, 10000.0, 40.0, 4096
BETA_FAST, BETA_SLOW, MSCALE, MSCALE_ALL = 32, 1, 0.707, 0.707

BF = mybir.dt.bfloat16
F32 = mybir.dt.float32
AX = mybir.AxisListType
AF = mybir.ActivationFunctionType


def _derived(cfg):
    d = dict(cfg)
    d["T_TOT"] = cfg["B"] * cfg["S"]
    d["T_LOC"] = d["T_TOT"] // cfg["N_CORES"]
    d["HPC"] = cfg["H"] // cfg["N_CORES"]
    d["KH"] = cfg["HID"] // 128
    d["KC"] = cfg["KV"] // 128
    d["TSUB"] = d["T_LOC"] // 128
    d["NCH"] = d["T_TOT"] // d["T_LOC"]
    # intermediate dim padded so each core gets ILJ chunks of 128
    n128 = (cfg["INTER"] + 127) // 128
    d["ILJ"] = (n128 + cfg["N_CORES"] - 1) // cfg["N_CORES"]   # chunks per core
    d["ILC"] = d["ILJ"] * 128                                   # inter rows per core
    d["INTER_PAD"] = d["ILC"] * cfg["N_CORES"]
    d["QTILES_B"] = cfg["S"] // 512
    d["KB_B"] = cfg["S"] // 128
    d["DQ"] = cfg["D_NOPE"] + cfg["D_ROPE"]
    # xnT + cnT + kpeT + cosLT + sinLT rows
    d["AGROWS"] = cfg["HID"] + cfg["KV"] + 2 * cfg["D_ROPE"]
    return d


# ---------------------------------------------------------------------------
def build_kernel(cfg):
    c = _derived(cfg)
    N = c["N_CORES"]
    HID, KV, DR, DN, DV = c["HID"], c["KV"], c["D_ROPE"], c["D_NOPE"], c["D_V"]
    TL, TT = c["T_LOC"], c["T_TOT"]
    KH, KC, TSUB, NCH = c["KH"], c["KC"], c["TSUB"], c["NCH"]
    HPC, DQ = c["HPC"], c["DQ"]
    QT_B, KB_B = c["QTILES_B"], c["KB_B"]
    B = c["B"]
    ILJ = c["ILJ"]
    HR = DR // 2
    AGR = c["AGROWS"]

    nc = bass.Bass()
    hid_e = nc.dram_tensor("hid", [TL, HID], BF, kind="ExternalInput")
    wqT_e = nc.dram_tensor("wqT", [HID, HPC * DQ], BF, kind="ExternalInput")
    wkvaT_e = nc.dram_tensor("wkvaT", [HID, KV + DR], BF, kind="ExternalInput")
    wbnT_e = nc.dram_tensor("wbnT", [KV, HPC * DN], BF, kind="ExternalInput")
    wbvT_e = nc.dram_tensor("wbvT", [KV, HPC * DV], BF, kind="ExternalInput")
    woT_e = nc.dram_tensor("woT", [HPC * DV, HID], BF, kind="ExternalInput")
    wg_e = nc.dram_tensor("wgp", [128, KH * ILJ * 128], BF, kind="ExternalInput")
    wu_e = nc.dram_tensor("wup", [128, KH * ILJ * 128], BF, kind="ExternalInput")
    wd_e = nc.dram_tensor("wdp", [128, ILJ * HID], BF, kind="ExternalInput")
    cosL_e = nc.dram_tensor("cosL", [TL, HR], BF, kind="ExternalInput")
    sinL_e = nc.dram_tensor("sinL", [TL, HR], BF, kind="ExternalInput")
    out_e = nc.dram_tensor("out", [TL, HID], BF, kind="ExternalOutput")
    probe = cfg.get("probe", False)
    if probe:
        p_agin_e = nc.dram_tensor("p_agin", [AGR, TL], BF, kind="ExternalOutput")
        p_x2_e = nc.dram_tensor("p_x2", [TL, HID], F32, kind="ExternalOutput")

    with tile.TileContext(nc) as tc:
        with (
            tc.tile_pool(name="dram", bufs=1, space="DRAM") as dram,
            tc.tile_pool(name="const", bufs=1) as const,
        ):
            agin = dram.tile([AGR, TL], BF, tag="agin", name="agin")
            agout = dram.tile([N * AGR, TL], BF, addr_space="Shared", tag="agout", name="agout")
            rs_in = dram.tile([TT, HID], BF, tag="rsin", name="rsin")
            rs_out = dram.tile([TL, HID], BF, tag="rsout", name="rsout")
            agin2 = dram.tile([HID, TL], BF, tag="agin2", name="agin2")
            agout2 = dram.tile([N * HID, TL], BF, addr_space="Shared", tag="agout2", name="agout2")
            rs2_in = dram.tile([TT, HID], BF, tag="rs2in", name="rs2in")
            rs2_out = dram.tile([TL, HID], BF, tag="rs2out", name="rs2out")
            x2_dr = dram.tile([TL, HID], F32, tag="x2dr", name="x2dr")

            ident = const.tile([128, 128], BF, tag="ident", name="ident")
            make_identity(nc, ident)
            eps_sb = const.tile([128, 1], F32, tag="eps", name="eps")
            nc.vector.memset(eps_sb[:], EPS)
            # mask[p, x] = 1.0 if x >= p + 384 else 0.0, generated on device
            mask_sb = const.tile([128, 896], BF, tag="mask", name="mask")
            nc.gpsimd.memset(mask_sb[:], 1.0)
            nc.gpsimd.affine_select(
                out=mask_sb[:], in_=mask_sb[:],
                compare_op=mybir.AluOpType.is_ge, fill=0.0,
                base=-384, pattern=[[1, 896]], channel_multiplier=-1,
            )
            cosT_sb = const.tile([HR, TT], F32, tag="cosT", name="cosT")
            sinT_sb = const.tile([HR, TT], F32, tag="sinT", name="sinT")
            cosL_sb = const.tile([128, TSUB, HR], F32, tag="cosL", name="cosL")
            nc.gpsimd.dma_start(cosL_sb[:], cosL_e.rearrange("(a p) r -> p a r", p=128))
            sinL_sb = const.tile([128, TSUB, HR], F32, tag="sinL", name="sinL")
            nc.gpsimd.dma_start(sinL_sb[:], sinL_e.rearrange("(a p) r -> p a r", p=128))

            # ============ phases 0-1: rms1, x^T, ckv, rms(c), rope(k_pe) =====
            with (
                tc.tile_pool(name="xnTp", bufs=1) as xnTp,
                tc.tile_pool(name="p0", bufs=2) as p0,
                tc.tile_pool(name="p01ps", bufs=2, space="PSUM") as p01ps,
            ):
                xnT = [xnTp.tile([128, TL], BF, tag=f"xnT{k}", name=f"xnT{k}") for k in range(KH)]
                xn_sb = []
                for t in range(TSUB):
                    ht = p0.tile([128, HID], BF, tag="hid0", name="hid0")
                    nc.sync.dma_start(ht[:], hid_e[t * 128:(t + 1) * 128, :])
                    sq = p0.tile([128, HID], F32, tag="sq", name="sq")
                    nc.vector.tensor_mul(sq[:], ht[:], ht[:])
                    ssum = p0.tile([128, 1], F32, tag="ssum", name="ssum")
                    nc.vector.reduce_sum(out=ssum[:], in_=sq[:], axis=AX.X)
                    rs = p0.tile([128, 1], F32, tag="rs", name="rs")
                    nc.scalar.activation(rs[:], ssum[:], AF.Sqrt, scale=1.0 / HID, bias=eps_sb[:])
                    nc.vector.reciprocal(rs[:], rs[:])
                    xt = p0.tile([128, HID], BF, tag="xn", name="xn", bufs=TSUB)
                    nc.vector.tensor_scalar_mul(xt[:], ht[:], rs[:])
                    xn_sb.append(xt)
                for t in range(TSUB):
                    for k in range(KH):
                        ps = p01ps.tile([128, 128], BF, tag="tr", name="tr")
                        nc.tensor.transpose(ps[:], xn_sb[t][:, k * 128:(k + 1) * 128], ident[:])
                        nc.scalar.copy(xnT[k][:, t * 128:(t + 1) * 128], ps[:])
                for k in range(KH):
                    nc.sync.dma_start(agin[k * 128:(k + 1) * 128, :], xnT[k][:])

                # phase 1
                wkva_sb = [p0.tile([128, KV + DR], BF, tag=f"wkva{k}", name=f"wkva{k}") for k in range(KH)]
                for k in range(KH):
                    nc.sync.dma_start(wkva_sb[k][:], wkvaT_e[k * 128:(k + 1) * 128, :])
                cnT_sb = [p0.tile([128, TL], BF, tag=f"cnT{j}", name=f"cnT{j}") for j in range(KC)]
                kpeT_loc = p0.tile([DR, TL], BF, tag="kpeT_loc", name="kpeT_loc")
                for t in range(TSUB):
                    ps_c = p01ps.tile([128, KV], F32, tag="psc", name="psc")
                    ps_p = p01ps.tile([128, DR], F32, tag="psp", name="psp")
                    for k in range(KH):
                        lq = xnT[k][:, t * 128:(t + 1) * 128]
                        nc.tensor.matmul(ps_c[:], lq, wkva_sb[k][:, :KV],
                                         start=(k == 0), stop=(k == KH - 1))
                        nc.tensor.matmul(ps_p[:], lq, wkva_sb[k][:, KV:],
                                         start=(k == 0), stop=(k == KH - 1))
                    sq = p0.tile([128, KV], F32, tag="sqc", name="sqc")
                    nc.scalar.activation(sq[:], ps_c[:], AF.Square)
                    ssum = p0.tile([128, 1], F32, tag="ssumc", name="ssumc")
                    nc.vector.reduce_sum(out=ssum[:], in_=sq[:], axis=AX.X)
                    rs = p0.tile([128, 1], F32, tag="rsc", name="rsc")
                    nc.scalar.activation(rs[:], ssum[:], AF.Sqrt, scale=1.0 / KV, bias=eps_sb[:])
                    nc.vector.reciprocal(rs[:], rs[:])
                    cn = p0.tile([128, KV], BF, tag="cn", name="cn")
                    nc.vector.tensor_scalar_mul(cn[:], ps_c[:], rs[:])
                    kp = p0.tile([128, DR], BF, tag="kp", name="kp")
                    a = p0.tile([128, HR], F32, tag="ra", name="ra")
                    b = p0.tile([128, HR], F32, tag="rb", name="rb")
                    cosl = cosL_sb[:, t, :]
                    sinl = sinL_sb[:, t, :]
                    nc.vector.tensor_mul(a[:], ps_p[:, :HR], cosl)
                    nc.vector.tensor_mul(b[:], ps_p[:, HR:], sinl)
                    nc.vector.tensor_sub(kp[:, :HR], a[:], b[:])
                    nc.vector.tensor_mul(a[:], ps_p[:, HR:], cosl)
                    nc.vector.tensor_mul(b[:], ps_p[:, :HR], sinl)
                    nc.vector.tensor_add(kp[:, HR:], a[:], b[:])
                    for j in range(KC):
                        ps = p01ps.tile([128, 128], BF, tag="tr", name="tr")
                        nc.tensor.transpose(ps[:], cn[:, j * 128:(j + 1) * 128], ident[:])
                        nc.scalar.copy(cnT_sb[j][:, t * 128:(t + 1) * 128], ps[:])
                    ps = p01ps.tile([128, 128], BF, tag="tr", name="tr")
                    nc.tensor.transpose(ps[:DR, :], kp[:], ident[:])
                    nc.scalar.copy(kpeT_loc[:, t * 128:(t + 1) * 128], ps[:DR, :])
                for j in range(KC):
                    nc.sync.dma_start(agin[HID + j * 128:HID + (j + 1) * 128, :], cnT_sb[j][:])
                nc.sync.dma_start(agin[HID + KV:HID + KV + DR, :], kpeT_loc[:])
                # ride cosL^T/sinL^T along the AllGather to avoid shipping
                # broadcast cosT/sinT tables to every core
                cosLT = p0.tile([HR, TL], BF, tag="cosLT", name="cosLT")
                sinLT = p0.tile([HR, TL], BF, tag="sinLT", name="sinLT")
                for t in range(TSUB):
                    cb = p0.tile([128, HR], BF, tag="cb", name="cb")
                    nc.scalar.copy(cb[:], cosL_sb[:, t, :])
                    ps = p01ps.tile([128, 128], BF, tag="tr", name="tr")
                    nc.tensor.transpose(ps[:HR, :], cb[:], ident[:])
                    nc.scalar.copy(cosLT[:, t * 128:(t + 1) * 128], ps[:HR, :])
                    sb2 = p0.tile([128, HR], BF, tag="sb2", name="sb2")
                    nc.scalar.copy(sb2[:], sinL_sb[:, t, :])
                    ps = p01ps.tile([128, 128], BF, tag="tr", name="tr")
                    nc.tensor.transpose(ps[:HR, :], sb2[:], ident[:])
                    nc.scalar.copy(sinLT[:, t * 128:(t + 1) * 128], ps[:HR, :])
                nc.sync.dma_start(agin[HID + KV + DR:HID + KV + DR + HR, :], cosLT[:])
                nc.sync.dma_start(agin[HID + KV + DR + HR:HID + KV + 2 * DR, :], sinLT[:])

            # ============ phase 2: AllGather ================================
            nc.gpsimd.collective_compute(
                "AllGather", mybir.AluOpType.bypass,
                replica_groups=[list(range(N))],
                ins=[agin.opt()], outs=[agout.opt()],
            )

            # assemble full cos/sin tables (f32) from the gathered slices
            for ch in range(NCH):
                base = ch * AGR + HID + KV + DR
                nc.gpsimd.dma_start(
                    cosT_sb[:, ch * TL:(ch + 1) * TL], agout[base:base + HR, :])
                nc.gpsimd.dma_start(
                    sinT_sb[:, ch * TL:(ch + 1) * TL], agout[base + HR:base + DR, :])

            if probe:
                with tc.tile_pool(name="prb0", bufs=2) as prb0:
                    for r in range(0, AGR, 128):
                        w = min(128, AGR - r)
                        pt_ = prb0.tile([128, TL], BF, tag="pgt", name="pgt")
                        nc.sync.dma_start(pt_[:w, :], agin[r:r + w, :])
                        nc.sync.dma_start(p_agin_e[r:r + w, :], pt_[:w, :])

            # ============ phases 3-5: attention ==============================
            with tc.tile_pool(name="asb", bufs=1) as asb:
                qnT = [asb.tile([128, TT], BF, tag=f"qnT{h}", name=f"qnT{h}") for h in range(HPC)]
                qpT = [asb.tile([DR, TT], BF, tag=f"qpT{h}", name=f"qpT{h}") for h in range(HPC)]
                knT = [asb.tile([128, TT], BF, tag=f"knT{h}", name=f"knT{h}") for h in range(HPC)]
                kpeT = asb.tile([DR, TT], BF, tag="kpeT", name="kpeT")
                v_sb = [asb.tile([128, TT // 128, DV + 4], BF, tag=f"v{h}", name=f"v{h}")
                        for h in range(HPC)]
                atT = [asb.tile([128, TT], BF, tag=f"atT{h}", name=f"atT{h}") for h in range(HPC)]

                with (
                    tc.tile_pool(name="p4w", bufs=1) as p4w,
                    tc.tile_pool(name="p4x", bufs=1) as p4x,
                    tc.tile_pool(name="p4", bufs=2) as p4,
                    tc.tile_pool(name="p4ps", bufs=2, space="PSUM") as p4ps,
                ):
                    wq_sb = [p4w.tile([128, HPC * DQ], BF, tag=f"wq{k}", name=f"wq{k}") for k in range(KH)]
                    for k in range(KH):
                        nc.sync.dma_start(wq_sb[k][:], wqT_e[k * 128:(k + 1) * 128, :])
                    wbn_sb = [p4w.tile([128, HPC * DN], BF, tag=f"wbn{j}", name=f"wbn{j}") for j in range(KC)]
                    wbv_sb = [p4w.tile([128, HPC * DV], BF, tag=f"wbv{j}", name=f"wbv{j}") for j in range(KC)]
                    for j in range(KC):
                        nc.sync.dma_start(wbn_sb[j][:], wbnT_e[j * 128:(j + 1) * 128, :])
                        nc.sync.dma_start(wbv_sb[j][:], wbvT_e[j * 128:(j + 1) * 128, :])

                    for ch in range(NCH):
                        nc.sync.dma_start(
                            kpeT[:, ch * TL:(ch + 1) * TL],
                            agout[ch * AGR + HID + KV: ch * AGR + HID + KV + DR, :])

                    for ch in range(NCH):
                        xch = []
                        for k in range(KH):
                            xt = p4x.tile([128, TL], BF, tag="xch", name="xch", bufs=KH + 4)
                            nc.sync.dma_start(
                                xt[:], agout[ch * AGR + k * 128: ch * AGR + (k + 1) * 128, :])
                            xch.append(xt)
                        cs = slice(ch * TL, (ch + 1) * TL)
                        for h in range(HPC):
                            ps_n = p4ps.tile([128, TL], F32, tag="qn", name="qn")
                            ps_p = p4ps.tile([DR, TL], F32, tag="qp", name="qp")
                            off = h * DQ
                            for k in range(KH):
                                nc.tensor.matmul(ps_n[:], wq_sb[k][:, off:off + DN], xch[k][:],
                                                 start=(k == 0), stop=(k == KH - 1))
                            for k in range(KH):
                                nc.tensor.matmul(ps_p[:], wq_sb[k][:, off + DN:off + DQ], xch[k][:],
                                                 start=(k == 0), stop=(k == KH - 1))
                            nc.scalar.copy(qnT[h][:, cs], ps_n[:])
                            a = p4.tile([HR, TL], F32, tag="qa", name="qa")
                            b = p4.tile([HR, TL], F32, tag="qb", name="qb")
                            cosc = cosT_sb[:, cs]
                            sinc = sinT_sb[:, cs]
                            nc.vector.tensor_mul(a[:], ps_p[:HR, :], cosc)
                            nc.vector.tensor_mul(b[:], ps_p[HR:, :], sinc)
                            nc.vector.tensor_sub(qpT[h][:HR, cs], a[:], b[:])
                            nc.vector.tensor_mul(a[:], ps_p[HR:, :], cosc)
                            nc.vector.tensor_mul(b[:], ps_p[:HR, :], sinc)
                            nc.vector.tensor_add(qpT[h][HR:, cs], a[:], b[:])

                    for ch in range(NCH):
                        cch = []
                        for j in range(KC):
                            ct = p4x.tile([128, TL], BF, tag="cch", name="cch", bufs=KC + 2)
                            nc.sync.dma_start(
                                ct[:], agout[ch * AGR + HID + j * 128: ch * AGR + HID + (j + 1) * 128, :])
                            cch.append(ct)
                        cs = slice(ch * TL, (ch + 1) * TL)
                        for h in range(HPC):
                            ps_k = p4ps.tile([128, TL], F32, tag="kn", name="kn")
                            for j in range(KC):
                                nc.tensor.matmul(ps_k[:], wbn_sb[j][:, h * DN:(h + 1) * DN], cch[j][:],
                                                 start=(j == 0), stop=(j == KC - 1))
                            nc.scalar.copy(knT[h][:, cs], ps_k[:])
                            for j4 in range(TL // 128):
                                ps_v = p4ps.tile([128, DV], F32, tag="pv", name="pv")
                                for j in range(KC):
                                    nc.tensor.matmul(ps_v[:], cch[j][:, j4 * 128:(j4 + 1) * 128],
                                                     wbv_sb[j][:, h * DV:(h + 1) * DV],
                                                     start=(j == 0), stop=(j == KC - 1))
                                kbt = ch * (TL // 128) + j4
                                nc.scalar.copy(v_sb[h][:, kbt, :DV], ps_v[:])
                                nc.vector.memset(v_sb[h][:, kbt, DV:DV + 1], 1.0)

                # ---------------- phase 5: attention -------------------------
                with (
                    tc.tile_pool(name="p5ps", bufs=2, space="PSUM") as p5ps,
                    tc.tile_pool(name="p5pv", bufs=2, space="PSUM") as p5pv,
                    tc.tile_pool(name="p5", bufs=2) as p5,
                    tc.tile_pool(name="prb", bufs=1) as prb,
                ):
                    for b in range(B):
                        for h in range(HPC):
                            for qt in range(QT_B):
                                qs = slice(b * cfg["S"] + qt * 512, b * cfg["S"] + qt * 512 + 512)
                                nkb = 4 * qt + 4
                                pt = []
                                for kb in range(nkb):
                                    kbg = b * KB_B + kb
                                    ks = slice(kbg * 128, kbg * 128 + 128)
                                    ps_s = p5ps.tile([128, 512], F32, tag="ps_s", name="ps_s")
                                    nc.tensor.matmul(ps_s[:], knT[h][:, ks], qnT[h][:, qs],
                                                     start=True, stop=False)
                                    nc.tensor.matmul(ps_s[:], kpeT[:, ks], qpT[h][:, qs],
                                                     start=False, stop=True)
                                    pb = prb.tile([128, 512], BF, tag="pb", name="pb", bufs=KB_B + 4)
                                    nc.scalar.activation(pb[:], ps_s[:], AF.Exp)
                                    delta = kb * 128 - qt * 512
                                    if delta >= 0:
                                        nc.vector.tensor_mul(
                                            pb[:], pb[:], mask_sb[:, 384 - delta:896 - delta])
                                    pt.append(pb)
                                for q4 in range(4):
                                    ps_av = p5pv.tile([128, DV + 4], F32, tag="ps_av", name="ps_av")
                                    for kb in range(nkb):
                                        kbt = b * KB_B + kb
                                        nc.tensor.matmul(
                                            ps_av[:, :DV + 1],
                                            pt[kb][:, q4 * 128:(q4 + 1) * 128],
                                            v_sb[h][:, kbt, :DV + 1],
                                            start=(kb == 0), stop=(kb == nkb - 1))
                                    recip = p5.tile([128, 1], F32, tag="recip", name="recip")
                                    nc.vector.reciprocal(recip[:], ps_av[:, DV:DV + 1])
                                    at = p5.tile([128, DV], BF, tag="at", name="at")
                                    nc.vector.tensor_scalar_mul(at[:], ps_av[:, :DV], recip[:])
                                    ps_t = p5ps.tile([128, 128], BF, tag="ps_t", name="ps_t")
                                    nc.tensor.transpose(ps_t[:DV, :], at[:], ident[:])
                                    qg = (b * cfg["S"] + qt * 512) // 128 + q4
                                    nc.scalar.copy(atT[h][:DV, qg * 128:(qg + 1) * 128], ps_t[:DV, :])

                # ============ phase 5b: row-parallel o_proj partials =============
                with (
                    tc.tile_pool(name="p6w", bufs=1) as p6w,
                    tc.tile_pool(name="p6", bufs=4) as p6,
                    tc.tile_pool(name="p6ps", bufs=4, space="PSUM") as p6ps,
                ):
                    wo_sb = [p6w.tile([128, HID], BF, tag=f"wo{j}", name=f"wo{j}") for j in range(HPC)]
                    for j in range(HPC):
                        nc.sync.dma_start(wo_sb[j][:], woT_e[j * DV:(j + 1) * DV, :])
                    for tq in range(TT // 128):
                        for nsl in range(HID // 512):
                            ps_o = p6ps.tile([128, 512], F32, tag="ps_o", name="ps_o")
                            for j in range(HPC):
                                nc.tensor.matmul(ps_o[:], atT[j][:DV, tq * 128:(tq + 1) * 128],
                                                 wo_sb[j][:, nsl * 512:(nsl + 1) * 512],
                                                 start=(j == 0), stop=(j == HPC - 1))
                            ob = p6.tile([128, 512], BF, tag="ob", name="ob")
                            nc.scalar.copy(ob[:], ps_o[:])
                            nc.sync.dma_start(
                                rs_in[tq * 128:(tq + 1) * 128, nsl * 512:(nsl + 1) * 512], ob[:])

            # ============ phase 6: ReduceScatter =============================
            nc.gpsimd.collective_compute(
                "ReduceScatter", mybir.AluOpType.add,
                replica_groups=[list(range(N))],
                ins=[rs_in.opt()], outs=[rs_out.opt()],
            )

            # ============ phase 7: residual, rms2, yT ========================
            with (
                tc.tile_pool(name="p7a", bufs=1) as p7a,
                tc.tile_pool(name="p7", bufs=2) as p7,
                tc.tile_pool(name="p7ps2", bufs=4, space="PSUM") as p7ps2,
            ):
                ynT = [p7a.tile([128, TL], BF, tag=f"ynT{k}", name=f"ynT{k}") for k in range(KH)]
                for t in range(TSUB):
                    hid_r = p7.tile([128, HID], BF, tag="hidr", name="hidr")
                    nc.sync.dma_start(hid_r[:], hid_e[t * 128:(t + 1) * 128, :])
                    rs_sb = p7.tile([128, HID], BF, tag="rssb", name="rssb")
                    nc.sync.dma_start(rs_sb[:], rs_out[t * 128:(t + 1) * 128, :])
                    x2 = p7.tile([128, HID], F32, tag="x2", name="x2")
                    nc.vector.tensor_add(x2[:], rs_sb[:], hid_r[:])
                    nc.sync.dma_start(x2_dr[t * 128:(t + 1) * 128, :], x2[:])
                    if probe:
                        nc.sync.dma_start(p_x2_e[t * 128:(t + 1) * 128, :], x2[:])
                    sq = p7.tile([128, HID], F32, tag="sq", name="sq")
                    nc.vector.tensor_mul(sq[:], x2[:], x2[:])
                    ssum = p7.tile([128, 1], F32, tag="ssum", name="ssum")
                    nc.vector.reduce_sum(out=ssum[:], in_=sq[:], axis=AX.X)
                    rs = p7.tile([128, 1], F32, tag="rs", name="rs")
                    nc.scalar.activation(rs[:], ssum[:], AF.Sqrt, scale=1.0 / HID, bias=eps_sb[:])
                    nc.vector.reciprocal(rs[:], rs[:])
                    yt = p7.tile([128, HID], BF, tag="yn", name="yn")
                    nc.vector.tensor_scalar_mul(yt[:], x2[:], rs[:])
                    for k in range(KH):
                        ps = p7ps2.tile([128, 128], BF, tag="tr", name="tr")
                        nc.tensor.transpose(ps[:], yt[:, k * 128:(k + 1) * 128], ident[:])
                        nc.scalar.copy(ynT[k][:, t * 128:(t + 1) * 128], ps[:])
                for k in range(KH):
                    nc.sync.dma_start(agin2[k * 128:(k + 1) * 128, :], ynT[k][:])

            # ============ phase 8a: AllGather(yT) ============================
            nc.gpsimd.collective_compute(
                "AllGather", mybir.AluOpType.bypass,
                replica_groups=[list(range(N))],
                ins=[agin2.opt()], outs=[agout2.opt()],
            )

            # ============ phase 8b: TP MLP over local intermediate slice =====
            with (
                tc.tile_pool(name="p8w", bufs=1) as p8w,
                tc.tile_pool(name="p8y", bufs=2) as p8y,
                tc.tile_pool(name="p8", bufs=3) as p8,
                tc.tile_pool(name="p8ps", bufs=2, space="PSUM") as p8ps,
            ):
                wg_sb = p8w.tile([128, KH * ILJ * 128], BF, tag="wg", name="wg")
                nc.sync.dma_start(wg_sb[:], wg_e[:])
                wu_sb = p8w.tile([128, KH * ILJ * 128], BF, tag="wu", name="wu")
                nc.sync.dma_start(wu_sb[:], wu_e[:])
                wd_sb = p8w.tile([128, ILJ * HID], BF, tag="wd", name="wd")
                nc.sync.dma_start(wd_sb[:], wd_e[:])
                for ch in range(NCH):
                    yblk = []
                    for k in range(KH):
                        yb = p8y.tile([128, TL], BF, tag="yb", name="yb", bufs=KH + 4)
                        nc.sync.dma_start(
                            yb[:], agout2[ch * HID + k * 128: ch * HID + (k + 1) * 128, :])
                        yblk.append(yb)
                    hblk = []
                    for j in range(ILJ):
                        ps_g = p8ps.tile([128, TL], F32, tag="psg", name="psg")
                        ps_u = p8ps.tile([128, TL], F32, tag="psu", name="psu")
                        for k in range(KH):
                            nc.tensor.matmul(ps_g[:], wg_sb[:, (k * ILJ + j) * 128:(k * ILJ + j + 1) * 128],
                                             yblk[k][:], start=(k == 0), stop=(k == KH - 1))
                        for k in range(KH):
                            nc.tensor.matmul(ps_u[:], wu_sb[:, (k * ILJ + j) * 128:(k * ILJ + j + 1) * 128],
                                             yblk[k][:], start=(k == 0), stop=(k == KH - 1))
                        sig = p8.tile([128, TL], BF, tag="sig", name="sig")
                        nc.scalar.activation(sig[:], ps_g[:], AF.Silu)
                        ht = p8.tile([128, TL], BF, tag="ht", name="ht", bufs=ILJ + 2)
                        nc.vector.tensor_mul(ht[:], sig[:], ps_u[:])
                        hblk.append(ht)
                    for tq in range(TL // 128):
                        for f in range(HID // 512):
                            ps_d = p8ps.tile([128, 512], F32, tag="psd", name="psd")
                            for j in range(ILJ):
                                nc.tensor.matmul(ps_d[:], hblk[j][:, tq * 128:(tq + 1) * 128],
                                                 wd_sb[:, j * HID + f * 512: j * HID + f * 512 + 512],
                                                 start=(j == 0), stop=(j == ILJ - 1))
                            ob = p8.tile([128, 512], BF, tag="ob", name="ob")
                            nc.scalar.copy(ob[:], ps_d[:])
                            nc.sync.dma_start(
                                rs2_in[ch * TL + tq * 128: ch * TL + (tq + 1) * 128,
                                       f * 512:(f + 1) * 512], ob[:])

            # ============ phase 8c: ReduceScatter(down partials) =============
            nc.gpsimd.collective_compute(
                "ReduceScatter", mybir.AluOpType.add,
                replica_groups=[list(range(N))],
                ins=[rs2_in.opt()], outs=[rs2_out.opt()],
            )

            # ============ phase 9: final residual ============================
            with tc.tile_pool(name="p9", bufs=2) as p9:
                for t in range(TSUB):
                    x2r = p9.tile([128, HID], F32, tag="x2r", name="x2r")
                    nc.sync.dma_start(x2r[:], x2_dr[t * 128:(t + 1) * 128, :])
                    mr = p9.tile([128, HID], BF, tag="mr", name="mr")
                    nc.sync.dma_start(mr[:], rs2_out[t * 128:(t + 1) * 128, :])
                    ot = p9.tile([128, HID], BF, tag="ot", name="ot")
                    nc.vector.tensor_add(ot[:], x2r[:], mr[:])
                    nc.sync.dma_start(out_e[t * 128:(t + 1) * 128, :], ot[:])
    return nc


# ---------------------------------------------------------------------------
# Host-side prep
# ---------------------------------------------------------------------------
def _yarn_tables(position_ids, d_rope):
    ar = np.arange(0, d_rope, 2, dtype=np.float32) / d_rope
    freq_extra = 1.0 / BASE ** ar
    freq_inter = 1.0 / (FACTOR * BASE ** ar)

    def corr_dim(num_rot):
        return d_rope * math.log(ORIG_MAX / (num_rot * 2 * math.pi)) / (2 * math.log(BASE))

    low = max(math.floor(corr_dim(BETA_FAST)), 0)
    high = min(math.ceil(corr_dim(BETA_SLOW)), d_rope - 1)
    hi = high + 0.001 if low == high else high
    ramp = np.clip((np.arange(d_rope // 2, dtype=np.float32) - low) / (hi - low), 0.0, 1.0)
    inv_freq_mask = 1.0 - ramp
    inv_freq = freq_inter * (1 - inv_freq_mask) + freq_extra * inv_freq_mask

    def get_mscale(s, m):
        return 1.0 if s <= 1 else 0.1 * m * math.log(s) + 1.0

    ms = get_mscale(FACTOR, MSCALE) / get_mscale(FACTOR, MSCALE_ALL)
    pos = np.asarray(position_ids).reshape(-1).astype(np.float32)
    fr = np.outer(pos, inv_freq)
    return (np.cos(fr) * ms).astype(np.float32), (np.sin(fr) * ms).astype(np.float32)


def _deint_perm(d):
    p = np.empty(d, np.int64)
    p[:d // 2] = 2 * np.arange(d // 2)
    p[d // 2:] = 2 * np.arange(d // 2) + 1
    return p


def prep_inputs(cfg, hidden_states, position_ids, Wq, Wkva, w_kvln, Wkvb, Wo,
                Wg, Wu, Wd, w_ln1, w_ln2):
    c = _derived(cfg)
    N, HPC = c["N_CORES"], c["HPC"]
    HID, KV, DR, DN, DV, DQ = c["HID"], c["KV"], c["D_ROPE"], c["D_NOPE"], c["D_V"], c["DQ"]
    TL, TT, KH = c["T_LOC"], c["T_TOT"], c["KH"]
    ILJ, ILC = c["ILJ"], c["ILC"]
    bf = ml_dtypes.bfloat16

    hid_flat = np.ascontiguousarray(hidden_states.reshape(TT, HID)).astype(bf)
    perm = _deint_perm(DR)
    scale = np.float32(DQ ** -0.5)

    Wq = Wq * w_ln1[None, :] * scale
    Wqh = Wq.reshape(cfg["H"], DQ, HID)
    Wqh = np.concatenate([Wqh[:, :DN], Wqh[:, DN:][:, perm]], axis=1)
    Wkva = Wkva * w_ln1[None, :]
    Wkva = np.concatenate([Wkva[:KV], Wkva[KV:][perm]], axis=0)
    wkvaT = np.ascontiguousarray(Wkva.T).astype(bf)
    Wkvb = Wkvb * w_kvln[None, :]
    Wkvbh = Wkvb.reshape(cfg["H"], DN + DV, KV)
    WoT_f = np.ascontiguousarray(Wo.T, dtype=np.float32)
    IP = c["INTER_PAD"]
    WgT = np.zeros((HID, IP), np.float32)
    WgT[:, :cfg["INTER"]] = (Wg * w_ln2[None, :]).T
    WuT = np.zeros((HID, IP), np.float32)
    WuT[:, :cfg["INTER"]] = (Wu * w_ln2[None, :]).T
    WdT = np.zeros((IP, HID), np.float32)
    WdT[:cfg["INTER"], :] = Wd.T

    cos_f, sin_f = _yarn_tables(position_ids, DR)

    in_maps = []
    for core in range(N):
        h0 = core * HPC
        wqT = np.ascontiguousarray(
            Wqh[h0:h0 + HPC].transpose(2, 0, 1).reshape(HID, HPC * DQ)).astype(bf)
        wbnT = np.ascontiguousarray(
            Wkvbh[h0:h0 + HPC, :DN].transpose(2, 0, 1).reshape(KV, HPC * DN)).astype(bf)
        wbvT = np.ascontiguousarray(
            Wkvbh[h0:h0 + HPC, DN:].transpose(2, 0, 1).reshape(KV, HPC * DV)).astype(bf)
        sl = slice(core * TL, (core + 1) * TL)
        isl = slice(core * ILC, (core + 1) * ILC)
        # [128 hid_in_part, KH*ILJ*128]: chunk (k,j) at cols (k*ILJ+j)*128
        wgp = np.ascontiguousarray(
            WgT[:, isl].reshape(KH, 128, ILJ * 128).transpose(1, 0, 2)
            .reshape(128, KH * ILJ * 128)).astype(bf)
        wup = np.ascontiguousarray(
            WuT[:, isl].reshape(KH, 128, ILJ * 128).transpose(1, 0, 2)
            .reshape(128, KH * ILJ * 128)).astype(bf)
        # [128 inter_part, ILJ*HID]: chunk j at cols j*HID
        wdp = np.ascontiguousarray(
            WdT[isl, :].reshape(ILJ, 128, HID).transpose(1, 0, 2)
            .reshape(128, ILJ * HID)).astype(bf)
        in_maps.append({
            "hid": hid_flat[sl],
            "wqT": wqT,
            "wkvaT": wkvaT,
            "wbnT": wbnT,
            "wbvT": wbvT,
            "woT": np.ascontiguousarray(WoT_f[h0 * DV:(h0 + HPC) * DV]).astype(bf),
            "wgp": wgp,
            "wup": wup,
            "wdp": wdp,
            "cosL": np.ascontiguousarray(cos_f[sl]).astype(bf),
            "sinL": np.ascontiguousarray(sin_f[sl]).astype(bf),
        })
    return in_maps


def run_cfg(cfg, nc, inputs_dict):
    from concourse.bass_utils import run_bass_kernel_spmd
    c = _derived(cfg)
    in_maps = prep_inputs(cfg, **inputs_dict)
    res = run_bass_kernel_spmd(nc, in_maps, list(range(cfg["N_CORES"])))
    out = np.concatenate(
        [res.results[i]["out"] for i in range(cfg["N_CORES"])], axis=0)
    return out.reshape(cfg["B"], cfg["S"], cfg["HID"]).astype(np.float32), res


_NC_CACHE = {}


def kernel(hidden_states, position_ids, Wq, Wkva, w_kvln, Wkvb, Wo, Wg, Wu, Wd,
           w_ln1, w_ln2):
    cfg = FULL_CFG
    if "full" not in _NC_CACHE:
        _NC_CACHE["full"] = build_kernel(cfg)
    out, _ = run_cfg(cfg, _NC_CACHE["full"], dict(
        hidden_states=np.asarray(hidden_states, np.float32),
        position_ids=np.asarray(position_ids),
        Wq=np.asarray(Wq, np.float32), Wkva=np.asarray(Wkva, np.float32),
        w_kvln=np.asarray(w_kvln, np.float32), Wkvb=np.asarray(Wkvb, np.float32),
        Wo=np.asarray(Wo, np.float32), Wg=np.asarray(Wg, np.float32),
        Wu=np.asarray(Wu, np.float32), Wd=np.asarray(Wd, np.float32),
        w_ln1=np.asarray(w_ln1, np.float32), w_ln2=np.asarray(w_ln2, np.float32)))
    return out

